# revision 20
# baseline (speedup 1.0000x reference)
"""Distributed Trainium2 Bass kernel for a transformer decoder layer.

Problem: nn_DECODERLAYER (B=2, S=2048, E=1024, H=16, Dh=64, MLP 4x, 3 LNs,
causal self-attention + causally-masked cross-attention to encoder K/V).

Sharding over 8 NeuronCores (SPMD, one program):
  - Tensor-parallel over heads: core c owns heads {2c, 2c+1} for both
    attentions (QKV / Qd projections are column-parallel).
  - Sequence-parallel for LN/residual/MLP regions: core c owns token rows
    [512c, 512c+512) of the flattened [B*S, E] token axis.
  - Collectives: AllToAll (ctx -> row shards) after each attention (1 MB),
    AllGather of y1^T (1 MB -> 8 MB) before the cross-attn Q projection.
    MLP is row-parallel (full W1/W2 per core, zero collectives).
Compute dtype: bf16 matmuls with f32 PSUM accumulation; LN/softmax math f32.
"""
import numpy as np
import ml_dtypes

B, S, E, H, Dh = 2, 2048, 1024, 16, 64
F = 4096
NCORES = 8
HPC = H // NCORES          # heads per core = 2
TOK = B * S                # 4096
LTOK = TOK // NCORES       # 512 local token rows
P = 128
NT = TOK // 512            # 8 token tiles of 512
EC = E // P                # 8 contraction chunks
LN_EPS = 1e-5
SCALE = 1.0 / np.sqrt(Dh)  # 0.125, exact in bf16

BF16 = ml_dtypes.bfloat16

_CACHE = {}


def _positional_encoding():
    # identical math to the reference (float32)
    i = np.arange(S, dtype=np.float32)[:, None]
    j = np.arange(E, dtype=np.float32)[None, :]
    even_arg = i / np.power(np.float32(10000.0), j / np.float32(E))
    odd_arg = np.cos(i / np.power(np.float32(10000.0), (j - 1.0) / np.float32(E)))
    return np.sin(np.where(np.arange(E)[None, :] % 2 == 0, even_arg, odd_arg)).astype(
        np.float32
    )


def _build_nc():
    import concourse.bacc as bacc
    import concourse.tile as tile
    import concourse.mybir as mybir
    from concourse.masks import make_identity
    from contextlib import ExitStack

    f32 = mybir.dt.float32
    bf16 = mybir.dt.bfloat16
    AF = mybir.ActivationFunctionType
    OP = mybir.AluOpType

    nc = bacc.Bacc("TRN2", target_bir_lowering=False, debug=False,
                   num_devices=NCORES)

    dram = lambda name, shape, dt: nc.dram_tensor(name, shape, dt,
                                                  kind="ExternalInput").ap()
    xT = dram("xT", [E, TOK], bf16)
    xrows = dram("xrows", [LTOK, E], f32)
    wqT = dram("wqT", [E, P], bf16)
    wkT = dram("wkT", [E, P], bf16)
    wvT = dram("wvT", [E, P], bf16)
    wqaT = dram("wqaT", [E, P], bf16)
    bq_d = dram("bq", [P, 1], f32)
    bk_d = dram("bk", [P, 1], f32)
    bqa_d = dram("bqa", [P, 1], f32)
    bv_d = dram("bv", [1, P], f32)
    kencT = dram("kencT", [HPC, B, Dh, S], bf16)
    venc = dram("venc", [HPC, B, S // P, P, Dh + 1], bf16)
    w1T = dram("w1T", [E, F], bf16)
    b1_d = dram("b1", [P, F // P], f32)
    w2T = dram("w2T", [F, E], bf16)
    b2_d = dram("b2", [1, E], f32)
    gb_d = dram("gb", [1, 6, E], f32)   # g1,be1,g2,be2,g3,be3
    masks_d = dram("masks", [4, P, 512], bf16)
    out_ext = nc.dram_tensor("out", [LTOK, E], f32, kind="ExternalOutput").ap()

    with tile.TileContext(nc) as tc, ExitStack() as top:
        # ---------- constants ----------
        const = top.enter_context(tc.tile_pool(name="const", bufs=1))
        wq_sb = const.tile([P, EC, P], bf16)
        nc.sync.dma_start(out=wq_sb[:], in_=wqT.rearrange("(c p) m -> p c m", p=P))
        wk_sb = const.tile([P, EC, P], bf16)
        nc.sync.dma_start(out=wk_sb[:], in_=wkT.rearrange("(c p) m -> p c m", p=P))
        wv_sb = const.tile([P, EC, P], bf16)
        nc.sync.dma_start(out=wv_sb[:], in_=wvT.rearrange("(c p) m -> p c m", p=P))
        bq_sb = const.tile([P, 1], f32)
        nc.sync.dma_start(out=bq_sb[:], in_=bq_d[:])
        bk_sb = const.tile([P, 1], f32)
        nc.sync.dma_start(out=bk_sb[:], in_=bk_d[:])
        bv_sb = const.tile([P, P], f32)
        nc.sync.dma_start(out=bv_sb[:], in_=bv_d[:].to_broadcast((P, P)))
        masks_sb = const.tile([P, 4, 512], bf16)
        nc.sync.dma_start(out=masks_sb[:], in_=masks_d.rearrange("j p q -> p j q"))
        ident = const.tile([P, P], bf16)
        make_identity(nc, ident[:])
        identf = const.tile([P, P], f32)
        make_identity(nc, identf[:])
        bqa_sb = const.tile([P, 1], f32)
        nc.sync.dma_start(out=bqa_sb[:], in_=bqa_d[:])
        wqa_sb = const.tile([P, EC, P], bf16)
        nc.sync.dma_start(out=wqa_sb[:], in_=wqaT.rearrange("(c p) m -> p c m", p=P))
        b1_sb = const.tile([P, F // P], f32)
        nc.sync.dma_start(out=b1_sb[:], in_=b1_d[:])
        b2_sb = const.tile([P, E], f32)
        nc.sync.dma_start(out=b2_sb[:], in_=b2_d[:].to_broadcast((P, E)))
        gb_sb = const.tile([P, 6, E], f32)
        nc.sync.dma_start(out=gb_sb[:], in_=gb_d[:].to_broadcast((P, 6, E)))
        eps_sb = const.tile([P, 1], f32)
        nc.vector.memset(eps_sb[:], LN_EPS)

        # persistent row-region tensors
        rows = top.enter_context(tc.tile_pool(name="rows", bufs=1))
        y1_sb = rows.tile([P, 4, E], f32)
        y2_sb = rows.tile([P, 4, E], f32)
        y2T_sb = rows.tile([P, EC, LTOK], bf16)
        res_sb = rows.tile([P, 4, E], f32)

        # DRAM bounce buffers for collectives
        dpool = top.enter_context(tc.tile_pool(name="dram", bufs=1, space="DRAM"))
        a2a1_in = dpool.tile([TOK, HPC * Dh], bf16)
        a2a1_out = dpool.tile([TOK, HPC * Dh], bf16)
        ag_in = dpool.tile([E, LTOK], bf16)
        ag_out = dpool.tile([NCORES, E, LTOK], bf16, addr_space="Shared")
        a2a2_in = dpool.tile([TOK, HPC * Dh], bf16)
        a2a2_out = dpool.tile([TOK, HPC * Dh], bf16)

        def ln_row_tile(y, dest, gi, bi):
            """LayerNorm on y [P, E] f32 (in-place scratch), write to dest."""
            stats = nc.vector
            st = ln_pool.tile([P, 2, stats.BN_STATS_DIM], f32)
            nc.vector.bn_stats(out=st[:, 0, :], in_=y[:, 0:512])
            nc.vector.bn_stats(out=st[:, 1, :], in_=y[:, 512:1024])
            mv = ln_pool.tile([P, stats.BN_AGGR_DIM], f32)
            nc.vector.bn_aggr(out=mv[:], in_=st[:])
            sd = ln_pool.tile([P, 1], f32)
            nc.scalar.activation(out=sd[:], in_=mv[:, 1:2], func=AF.Sqrt,
                                 bias=eps_sb[:])
            rstd = ln_pool.tile([P, 1], f32)
            nc.vector.reciprocal(out=rstd[:], in_=sd[:])
            nc.vector.tensor_scalar(out=y[:], in0=y[:], scalar1=mv[:, 0:1],
                                    scalar2=rstd[:], op0=OP.subtract, op1=OP.mult)
            nc.vector.tensor_tensor(out=y[:], in0=y[:], in1=gb_sb[:, gi, :],
                                    op=OP.mult)
            nc.vector.tensor_tensor(out=dest, in0=y[:], in1=gb_sb[:, bi, :],
                                    op=OP.add)

        # ==== Phase B+C interleaved: QKV projections + causal self-attention ====
        # Emission order: per token-tile tt: QKV(tt), then the attention
        # groups (b=tt//4, qt=tt%4, h=0,1) whose inputs are now complete.
        # PE-heavy projections overlap ACT-heavy softmax exp.
        with ExitStack() as ph:
            qkv = ph.enter_context(tc.tile_pool(name="qkv", bufs=1))
            QT_sb = qkv.tile([P, NT, 512], bf16)
            KT_sb = qkv.tile([P, NT, 512], bf16)
            V_sb = qkv.tile([P, HPC, TOK // P, Dh + 1], bf16)
            asm1_sb = qkv.tile([P, TOK // P, HPC * Dh], bf16)
            nc.vector.memset(V_sb[:, :, :, Dh:Dh + 1], 1.0)

            xs = ph.enter_context(tc.tile_pool(name="xs", bufs=2))
            attn = ph.enter_context(tc.tile_pool(name="attnC", bufs=2))
            pss = ph.enter_context(tc.tile_pool(name="psS", bufs=2, space="PSUM"))
            mix = ph.enter_context(tc.tile_pool(name="psMix", bufs=1, space="PSUM"))
            rp = ph.enter_context(tc.tile_pool(name="rpC", bufs=4))

            def qkv_tile(tt):
                xt = xs.tile([P, EC, 512], bf16, tag="xt", name="xt")
                nc.sync.dma_start(
                    out=xt[:],
                    in_=xT.rearrange("(c p) t -> p c t", p=P)[:, :, tt * 512:(tt + 1) * 512])
                qps = mix.tile([P, 512], f32, tag="qk", name="qps", bufs=2)
                for c in range(EC):
                    nc.tensor.matmul(qps[:], wq_sb[:, c, :], xt[:, c, :],
                                     start=(c == 0), stop=(c == EC - 1))
                # QT = (psum + bq) * 0.125  (fold softmax scale into Q)
                nc.vector.tensor_scalar(out=QT_sb[:, tt, :], in0=qps[:],
                                        scalar1=bq_sb[:], scalar2=SCALE,
                                        op0=OP.add, op1=OP.mult)
                kps = mix.tile([P, 512], f32, tag="qk", name="kps", bufs=2)
                for c in range(EC):
                    nc.tensor.matmul(kps[:], wk_sb[:, c, :], xt[:, c, :],
                                     start=(c == 0), stop=(c == EC - 1))
                nc.vector.tensor_scalar_add(out=KT_sb[:, tt, :], in0=kps[:],
                                            scalar1=bk_sb[:])
                for v in range(4):
                    kc = tt * 4 + v
                    vps = mix.tile([P, P], f32, tag="qk", name="vps", bufs=2)
                    for c in range(EC):
                        nc.tensor.matmul(vps[:], xt[:, c, v * P:(v + 1) * P],
                                         wv_sb[:, c, :],
                                         start=(c == 0), stop=(c == EC - 1))
                    for h in range(HPC):
                        nc.vector.tensor_tensor(
                            out=V_sb[:, h, kc, 0:Dh],
                            in0=vps[:, h * Dh:(h + 1) * Dh],
                            in1=bv_sb[:, h * Dh:(h + 1) * Dh],
                            op=OP.add)

            def attn_group(b, h, qt):
                tt_q = 4 * b + qt
                nkc = 4 * (qt + 1)
                pbuf = attn.tile([P, 8, 2, 512], bf16, tag="p", name="pbuf")

                def evict(cx, qs):
                    rcp = rp.tile([P, 1], f32, tag="rcp", name="rcp")
                    nc.vector.reciprocal(out=rcp[:], in_=cx[:, Dh:Dh + 1])
                    nc.vector.tensor_scalar_mul(
                        out=asm1_sb[:, tt_q * 4 + qs, h * Dh:(h + 1) * Dh],
                        in0=cx[:, 0:Dh], scalar1=rcp[:])

                # pass 1: scores + exp + PV for qs 0/1
                cxa = mix.tile([P, Dh + 1], f32, tag="ctxA", name="cxa")
                cxb = mix.tile([P, Dh + 1], f32, tag="ctxB", name="cxb")
                for pr in range(nkc // 2):
                    kc0 = 2 * pr
                    st = pss.tile([P, 2, 512], f32, tag="st", name="st")
                    for d in range(2):
                        kc = kc0 + d
                        tt_k = 4 * b + kc // 4
                        off = (kc % 4) * P
                        nc.tensor.matmul(
                            st[:, d, :],
                            KT_sb[h * Dh:(h + 1) * Dh, tt_k, off:off + P],
                            QT_sb[h * Dh:(h + 1) * Dh, tt_q, :],
                            start=True, stop=True)
                    nc.scalar.activation(out=pbuf[:, pr, :, :], in_=st[:],
                                         func=AF.Exp)
                    if kc0 >= 4 * qt:
                        j = kc0 - 4 * qt
                        nc.vector.tensor_tensor(out=pbuf[:, pr, :, :],
                                                in0=pbuf[:, pr, :, :],
                                                in1=masks_sb[:, j:j + 2, :],
                                                op=OP.mult)
                    for d in range(2):
                        kc = kc0 + d
                        nc.tensor.matmul(
                            cxa[:], pbuf[:, pr, d, 0:P],
                            V_sb[:, h, b * 16 + kc, :],
                            start=(kc == 0), stop=(kc == nkc - 1))
                        nc.tensor.matmul(
                            cxb[:], pbuf[:, pr, d, P:2 * P],
                            V_sb[:, h, b * 16 + kc, :],
                            start=(kc == 0), stop=(kc == nkc - 1))
                evict(cxa, 0)
                evict(cxb, 1)
                # pass 2: PV for qs 2/3 from the buffered probabilities
                cxa = mix.tile([P, Dh + 1], f32, tag="ctxA", name="cxa2")
                cxb = mix.tile([P, Dh + 1], f32, tag="ctxB", name="cxb2")
                for pr in range(nkc // 2):
                    for d in range(2):
                        kc = 2 * pr + d
                        nc.tensor.matmul(
                            cxa[:], pbuf[:, pr, d, 2 * P:3 * P],
                            V_sb[:, h, b * 16 + kc, :],
                            start=(kc == 0), stop=(kc == nkc - 1))
                        nc.tensor.matmul(
                            cxb[:], pbuf[:, pr, d, 3 * P:4 * P],
                            V_sb[:, h, b * 16 + kc, :],
                            start=(kc == 0), stop=(kc == nkc - 1))
                evict(cxa, 2)
                evict(cxb, 3)

            for tt in range(NT):
                qkv_tile(tt)
                for h in range(HPC):
                    attn_group(tt // 4, h, tt % 4)

            nc.sync.dma_start(out=a2a1_in[:].rearrange("(c p) d -> p c d", p=P),
                              in_=asm1_sb[:])
        nc.gpsimd.collective_compute(
            "AllToAll", OP.bypass, replica_groups=[list(range(NCORES))],
            ins=[a2a1_in[:].opt()], outs=[a2a1_out[:].opt()])

        # ====== Phase D: LN1 rows, y1, y1^T, AllGather y1^T ======
        with ExitStack() as ph:
            ln_pool = ph.enter_context(tc.tile_pool(name="lnD", bufs=3))
            work = ph.enter_context(tc.tile_pool(name="workD", bufs=3))
            pst = ph.enter_context(tc.tile_pool(name="psT", bufs=2, space="PSUM"))
            xr = ph.enter_context(tc.tile_pool(name="xr", bufs=1))
            xrows_sb = xr.tile([P, 4, E], f32)
            nc.sync.dma_start(out=xrows_sb[:],
                              in_=xrows.rearrange("(r p) e -> p r e", p=P))
            y1T_sb = xr.tile([P, EC, LTOK], bf16)
            ctx_v = a2a1_out[:].rearrange("(i t) d -> i t d", i=NCORES)
            for rt in range(4):
                ln_in = work.tile([P, NCORES, HPC * Dh], bf16, tag="lnin")
                nc.sync.dma_start(
                    out=ln_in[:],
                    in_=ctx_v[:, rt * P:(rt + 1) * P, :].rearrange("i p d -> p i d"))
                y = work.tile([P, E], f32, tag="y")
                nc.vector.tensor_tensor(out=y[:],
                                        in0=ln_in[:].rearrange("p i d -> p (i d)"),
                                        in1=xrows_sb[:, rt, :], op=OP.add)
                ln_row_tile(y, y1_sb[:, rt, :], 0, 1)
                ybf = work.tile([P, E], bf16, tag="ybf")
                nc.scalar.copy(out=ybf[:], in_=y1_sb[:, rt, :])
                for c in range(EC):
                    tp = pst.tile([P, P], bf16, tag="tp")
                    nc.tensor.transpose(tp[:], ybf[:, c * P:(c + 1) * P], ident[:])
                    nc.scalar.copy(out=y1T_sb[:, c, rt * P:(rt + 1) * P],
                                   in_=tp[:])
            nc.sync.dma_start(out=ag_in[:].rearrange("(c p) t -> p c t", p=P),
                              in_=y1T_sb[:])
        nc.gpsimd.collective_compute(
            "AllGather", OP.bypass, replica_groups=[list(range(NCORES))],
            ins=[ag_in[:].opt()], outs=[ag_out[:].opt()])

        # ==== Phase E+F interleaved: Qd projection + causal cross-attention ====
        with ExitStack() as ph:
            qd = ph.enter_context(tc.tile_pool(name="qd", bufs=1))
            QdT_sb = qd.tile([P, NT, 512], bf16)
            kenc_sb = qd.tile([P, B, S], bf16)
            venc_sb = qd.tile([P, HPC, B, S // P, Dh + 1], bf16)
            asm2_sb = qd.tile([P, TOK // P, HPC * Dh], bf16)
            for h in range(HPC):
                nc.sync.dma_start(out=kenc_sb[h * Dh:(h + 1) * Dh, :, :],
                                  in_=kencT[h].rearrange("b d s -> d b s"))
                nc.sync.dma_start(out=venc_sb[:, h, :, :, :],
                                  in_=venc[h].rearrange("b c p d -> p b c d"))

            ags = ph.enter_context(tc.tile_pool(name="ags", bufs=2))
            attn2 = ph.enter_context(tc.tile_pool(name="attnF", bufs=2))
            pss2 = ph.enter_context(tc.tile_pool(name="psS2", bufs=2, space="PSUM"))
            mix2 = ph.enter_context(tc.tile_pool(name="psMix2", bufs=1, space="PSUM"))
            rp2 = ph.enter_context(tc.tile_pool(name="rp2", bufs=4))

            def qd_tile(tt):
                agt = ags.tile([P, EC, 512], bf16, tag="agt", name="agt")
                nc.sync.dma_start(out=agt[:],
                                  in_=ag_out[tt].rearrange("(c p) t -> p c t", p=P))
                qps = mix2.tile([P, 512], f32, tag="qdps", name="qdps", bufs=2)
                for c in range(EC):
                    nc.tensor.matmul(qps[:], wqa_sb[:, c, :], agt[:, c, :],
                                     start=(c == 0), stop=(c == EC - 1))
                nc.vector.tensor_scalar(out=QdT_sb[:, tt, :], in0=qps[:],
                                        scalar1=bqa_sb[:], scalar2=SCALE,
                                        op0=OP.add, op1=OP.mult)

            def attn2_group(b, h, qt):
                tt_q = 4 * b + qt
                nkc = 4 * (qt + 1)
                pbuf = attn2.tile([P, 8, 2, 512], bf16, tag="p2", name="pbuf2")

                def evict(cx, qs):
                    rcp = rp2.tile([P, 1], f32, tag="rcp2", name="rcp2")
                    nc.vector.reciprocal(out=rcp[:], in_=cx[:, Dh:Dh + 1])
                    nc.vector.tensor_scalar_mul(
                        out=asm2_sb[:, tt_q * 4 + qs, h * Dh:(h + 1) * Dh],
                        in0=cx[:, 0:Dh], scalar1=rcp[:])

                cxa = mix2.tile([P, Dh + 1], f32, tag="ctxA2", name="cxa_")
                cxb = mix2.tile([P, Dh + 1], f32, tag="ctxB2", name="cxb_")
                for pr in range(nkc // 2):
                    kc0 = 2 * pr
                    st = pss2.tile([P, 2, 512], f32, tag="st2", name="st2")
                    for d in range(2):
                        kc = kc0 + d
                        nc.tensor.matmul(
                            st[:, d, :],
                            kenc_sb[h * Dh:(h + 1) * Dh, b, kc * P:(kc + 1) * P],
                            QdT_sb[h * Dh:(h + 1) * Dh, tt_q, :],
                            start=True, stop=True)
                    nc.scalar.activation(out=pbuf[:, pr, :, :], in_=st[:],
                                         func=AF.Exp)
                    if kc0 >= 4 * qt:
                        j = kc0 - 4 * qt
                        nc.vector.tensor_tensor(out=pbuf[:, pr, :, :],
                                                in0=pbuf[:, pr, :, :],
                                                in1=masks_sb[:, j:j + 2, :],
                                                op=OP.mult)
                    for d in range(2):
                        kc = kc0 + d
                        nc.tensor.matmul(
                            cxa[:], pbuf[:, pr, d, 0:P],
                            venc_sb[:, h, b, kc, :],
                            start=(kc == 0), stop=(kc == nkc - 1))
                        nc.tensor.matmul(
                            cxb[:], pbuf[:, pr, d, P:2 * P],
                            venc_sb[:, h, b, kc, :],
                            start=(kc == 0), stop=(kc == nkc - 1))
                evict(cxa, 0)
                evict(cxb, 1)
                cxa = mix2.tile([P, Dh + 1], f32, tag="ctxA2", name="cxa2_")
                cxb = mix2.tile([P, Dh + 1], f32, tag="ctxB2", name="cxb2_")
                for pr in range(nkc // 2):
                    for d in range(2):
                        kc = 2 * pr + d
                        nc.tensor.matmul(
                            cxa[:], pbuf[:, pr, d, 2 * P:3 * P],
                            venc_sb[:, h, b, kc, :],
                            start=(kc == 0), stop=(kc == nkc - 1))
                        nc.tensor.matmul(
                            cxb[:], pbuf[:, pr, d, 3 * P:4 * P],
                            venc_sb[:, h, b, kc, :],
                            start=(kc == 0), stop=(kc == nkc - 1))
                evict(cxa, 2)
                evict(cxb, 3)

            for tt in range(NT):
                qd_tile(tt)
                for h in range(HPC):
                    attn2_group(tt // 4, h, tt % 4)

            nc.sync.dma_start(out=a2a2_in[:].rearrange("(c p) d -> p c d", p=P),
                              in_=asm2_sb[:])
        nc.gpsimd.collective_compute(
            "AllToAll", OP.bypass, replica_groups=[list(range(NCORES))],
            ins=[a2a2_in[:].opt()], outs=[a2a2_out[:].opt()])

        # ====== Phase G: LN2 rows, y2, y2^T ======
        with ExitStack() as ph:
            ln_pool = ph.enter_context(tc.tile_pool(name="lnG", bufs=3))
            work = ph.enter_context(tc.tile_pool(name="workG", bufs=3))
            pst = ph.enter_context(tc.tile_pool(name="psT2", bufs=2, space="PSUM"))
            ctx_v = a2a2_out[:].rearrange("(i t) d -> i t d", i=NCORES)
            for rt in range(4):
                ln_in = work.tile([P, NCORES, HPC * Dh], bf16, tag="lnin2")
                nc.sync.dma_start(
                    out=ln_in[:],
                    in_=ctx_v[:, rt * P:(rt + 1) * P, :].rearrange("i p d -> p i d"))
                y = work.tile([P, E], f32, tag="y2w")
                nc.vector.tensor_tensor(out=y[:],
                                        in0=ln_in[:].rearrange("p i d -> p (i d)"),
                                        in1=y1_sb[:, rt, :], op=OP.add)
                ln_row_tile(y, y2_sb[:, rt, :], 2, 3)
                ybf = work.tile([P, E], bf16, tag="ybf2")
                nc.scalar.copy(out=ybf[:], in_=y2_sb[:, rt, :])
                for c in range(EC):
                    tp = pst.tile([P, P], bf16, tag="tp2")
                    nc.tensor.transpose(tp[:], ybf[:, c * P:(c + 1) * P], ident[:])
                    nc.scalar.copy(out=y2T_sb[:, c, rt * P:(rt + 1) * P],
                                   in_=tp[:])

        # ====== Phase H: MLP (row-parallel) + LN3 + output ======
        with ExitStack() as ph:
            hp = ph.enter_context(tc.tile_pool(name="hT", bufs=1))
            hT_sb = hp.tile([P, F // P, LTOK], bf16)
            w1s = ph.enter_context(tc.tile_pool(name="w1s", bufs=3))
            psh = ph.enter_context(tc.tile_pool(name="psH", bufs=2, space="PSUM"))
            for m in range(F // P):
                w1t = w1s.tile([P, EC, P], bf16, tag="w1t")
                nc.sync.dma_start(
                    out=w1t[:],
                    in_=w1T.rearrange("(c p) f -> p c f", p=P)[:, :, m * P:(m + 1) * P])
                hps = psh.tile([P, 512], f32, tag="hps")
                for c in range(EC):
                    nc.tensor.matmul(hps[:], w1t[:, c, :], y2T_sb[:, c, :],
                                     start=(c == 0), stop=(c == EC - 1))
                nc.scalar.activation(out=hT_sb[:, m, :], in_=hps[:], func=AF.Gelu,
                                     bias=b1_sb[:, m:m + 1])

            # y2b = y2 + b2 (residual + output bias, done once)
            for rt in range(4):
                nc.vector.tensor_tensor(out=y2_sb[:, rt, :], in0=y2_sb[:, rt, :],
                                        in1=b2_sb[:], op=OP.add)

            w2s = ph.enter_context(tc.tile_pool(name="w2s", bufs=3))
            psm = ph.enter_context(tc.tile_pool(name="psM", bufs=1, space="PSUM"))
            for eh in range(2):
                mps = psm.tile([P, 4, 512], f32, tag="mps")
                for m in range(F // P):
                    w2t = w2s.tile([P, 512], bf16, tag="w2t")
                    nc.sync.dma_start(
                        out=w2t[:],
                        in_=w2T.rearrange("(m p) e -> p m e", p=P)[:, m,
                                                                  eh * 512:(eh + 1) * 512])
                    for tq in range(4):
                        nc.tensor.matmul(mps[:, tq, :],
                                         hT_sb[:, m, tq * P:(tq + 1) * P], w2t[:],
                                         start=(m == 0), stop=(m == F // P - 1))
                for tq in range(4):
                    nc.vector.tensor_tensor(
                        out=res_sb[:, tq, eh * 512:(eh + 1) * 512],
                        in0=mps[:, tq, :], in1=y2_sb[:, tq, eh * 512:(eh + 1) * 512],
                        op=OP.add)

            ln_pool = ph.enter_context(tc.tile_pool(name="lnH", bufs=3))
            outw = ph.enter_context(tc.tile_pool(name="outw", bufs=2))
            for tq in range(4):
                orow = outw.tile([P, E], f32, tag="orow")
                ln_row_tile(res_sb[:, tq, :], orow[:], 4, 5)
                nc.sync.dma_start(out=out_ext[tq * P:(tq + 1) * P, :], in_=orow[:])

    nc.finalize()
    return nc


def _stage_inputs(inputs):
    """Host-side sharding: build per-core in_maps."""
    x = np.asarray(inputs["inputs"], np.float32)
    kenc = np.asarray(inputs["k_from_encoder"], np.float32)
    venc = np.asarray(inputs["v_from_encoder"], np.float32)
    Wq, bq = np.asarray(inputs["Wqm"], np.float32), np.asarray(inputs["bqm"], np.float32)
    Wk, bk = np.asarray(inputs["Wkm"], np.float32), np.asarray(inputs["bkm"], np.float32)
    Wv, bv = np.asarray(inputs["Wvm"], np.float32), np.asarray(inputs["bvm"], np.float32)
    Wqa, bqa = np.asarray(inputs["Wqa"], np.float32), np.asarray(inputs["bqa"], np.float32)
    W1, b1 = np.asarray(inputs["W1"], np.float32), np.asarray(inputs["b1"], np.float32)
    W2, b2 = np.asarray(inputs["W2"], np.float32), np.asarray(inputs["b2"], np.float32)

    pe = _positional_encoding()
    xf = (x + pe[None]).reshape(TOK, E)          # [4096, 1024] f32
    xT = np.ascontiguousarray(xf.T).astype(BF16)  # [1024, 4096]

    # causal diagonal-block masks [4, 128, 512]
    kl = np.arange(P)[:, None]
    ql = np.arange(512)[None, :]
    masks = np.stack([(P * j + kl <= ql) for j in range(4)]).astype(BF16)

    gb = np.stack([np.asarray(inputs[k], np.float32)
                   for k in ("g1", "be1", "g2", "be2", "g3", "be3")])[None]

    w1T = np.ascontiguousarray(W1.T).astype(BF16)          # [E, F]
    b1_t = np.ascontiguousarray(b1.reshape(F // P, P).T)   # [P, F//P]
    w2T = np.ascontiguousarray(W2.T).astype(BF16)          # [F, E]

    in_maps = []
    for c in range(NCORES):
        hs = slice(c * HPC * Dh, (c + 1) * HPC * Dh)       # head-col slice (128)
        ts = slice(c * LTOK, (c + 1) * LTOK)               # token-row slice (512)
        kencT_c = np.ascontiguousarray(
            kenc[:, :, c * HPC:(c + 1) * HPC, :].transpose(2, 0, 3, 1)).astype(BF16)
        v_c = venc[:, :, c * HPC:(c + 1) * HPC, :].transpose(2, 0, 1, 3)  # [h,b,S,Dh]
        v_aug = np.ones((HPC, B, S, Dh + 1), np.float32)
        v_aug[..., :Dh] = v_c
        v_aug = v_aug.reshape(HPC, B, S // P, P, Dh + 1).astype(BF16)
        in_maps.append({
            "xT": xT,
            "xrows": np.ascontiguousarray(xf[ts]),
            "wqT": np.ascontiguousarray(Wq[hs].T).astype(BF16),
            "wkT": np.ascontiguousarray(Wk[hs].T).astype(BF16),
            "wvT": np.ascontiguousarray(Wv[hs].T).astype(BF16),
            "wqaT": np.ascontiguousarray(Wqa[hs].T).astype(BF16),
            "bq": np.ascontiguousarray(bq[hs][:, None]),
            "bk": np.ascontiguousarray(bk[hs][:, None]),
            "bqa": np.ascontiguousarray(bqa[hs][:, None]),
            "bv": np.ascontiguousarray(bv[hs][None, :]),
            "kencT": kencT_c,
            "venc": np.ascontiguousarray(v_aug),
            "w1T": w1T,
            "b1": b1_t,
            "w2T": w2T,
            "b2": np.ascontiguousarray(b2[None, :]),
            "gb": gb,
            "masks": masks,
        })
    return in_maps


def kernel(**inputs):
    from concourse.bass_utils import run_bass_kernel_spmd

    if "nc" not in _CACHE:
        _CACHE["nc"] = _build_nc()
    nc = _CACHE["nc"]
    in_maps = _stage_inputs(inputs)
    res = run_bass_kernel_spmd(nc, in_maps, core_ids=list(range(NCORES)))
    out = np.concatenate([res.results[c]["out"] for c in range(NCORES)], axis=0)
    return out.reshape(B, S, E).astype(np.float32)
